# revision 42
# baseline (speedup 1.0000x reference)
"""Trainium2 Bass kernel for a 2-layer GatedGraphConv encoder (9 convs, 18
message-passing + GRU steps) on N=50000 nodes, E=800000 edges, C=128.

Strategy (8 NeuronCores, SPMD single program):
  - Nodes are block-sharded: core c owns dst rows [c*6250, (c+1)*6250).
  - Edges live on the core that owns their dst.  Per core, edges are bucketed
    into 32-dst windows and padded to 128-edge chunks (chunk counts are the
    max over cores so the shared program works for every core).
  - Per layer: each core computes its m = x @ W slice with the tensor engine,
    an AllGather materializes the full message table in DRAM, dma_gather
    pulls the per-edge message rows into SBUF, and one 128x32 matmul per
    chunk (selector = ew-scaled one-hot of dst-in-window) accumulates
    agg^T = sum_e ew_e * m[src_e] directly in PSUM, feature-major.
  - The GRU update runs entirely feature-major: gate matmuls stream the
    512-node PSUM group through pre-transposed GRU weights; sigmoids/tanh on
    the scalar engine (per-partition bias = per-channel bias), elementwise on
    the vector engine.  x^T stays resident in SBUF across all 18 layers.
"""
import numpy as np

import concourse.bacc as bacc
import concourse.mybir as mybir
import concourse.tile as tile
from concourse import bass_utils

N = 50000
C = 128
NCORES = 8
NPC = N // NCORES            # 6250
WIN = 64                     # dst nodes per selector window
GRP = 8                      # windows per 512-col PSUM group
CHUNK = 128                  # edges per selector matmul
HL = 3200                    # local-node split: table A = locals [0,HL) of
                             # every core, table B = the rest; both fit int16
NWIN = (NPC + WIN - 1) // WIN            # 196
NGRP = (NWIN + GRP - 1) // GRP           # 13
NLAYERS = 18
SUB = 32                    # max chunks per gather instruction / G tile

F32 = mybir.dt.float32
F32R = mybir.dt.float32r        # relaxed fp32: 1 cyc/row when moving >= 256
I16 = mybir.dt.int16
import os
MSG_BF16 = os.environ.get("K_MSG_BF16", "0") == "1"   # msg table/gather/sel
GATE_BF16 = os.environ.get("K_GATE_BF16", "0") == "1" # GRU gate matmuls
MDT = mybir.dt.bfloat16 if MSG_BF16 else F32
import ml_dtypes
MDT_NP = ml_dtypes.bfloat16 if MSG_BF16 else np.float32
GDT = mybir.dt.bfloat16 if GATE_BF16 else F32
GDT_NP = ml_dtypes.bfloat16 if GATE_BF16 else np.float32
# 0: stream sel from DRAM; 1: per-chunk on-device gen; 2: batched gen
SEL_MODE = int(os.environ.get("K_SEL_ONDEV", "0"))
SEL_ONDEV = SEL_MODE > 0


# --------------------------------------------------------------------------
# host-side preprocessing
# --------------------------------------------------------------------------

def preprocess(edge_index, edge_attr):
    src = np.asarray(edge_index[0], dtype=np.int64)
    dst = np.asarray(edge_index[1], dtype=np.int64)
    ew = np.asarray(edge_attr, dtype=np.float32)
    E = src.shape[0]

    owner = dst // NPC
    dst_local = dst - owner * NPC
    win = dst_local // WIN
    src_owner = src // NPC
    src_local = src - src_owner * NPC
    half = (src_local >= HL).astype(np.int64)
    tab_idx = np.where(half == 0, src_owner * HL + src_local,
                       src_owner * (NPC - HL) + (src_local - HL))

    counts = np.zeros((NCORES, NWIN, 2), dtype=np.int64)
    np.add.at(counts, (owner, win, half), 1)
    K = (counts.max(axis=0) + CHUNK - 1) // CHUNK          # [NWIN, 2]
    # Every (window, half) needs >= 1 chunk: each half accumulates in its own
    # PSUM bank and agg = lo + hi, so every column must be written in both.
    K = np.maximum(K, 1)

    order = np.lexsort((half, win, owner))
    so, sw, sh = owner[order], win[order], half[order]
    ssrc, sdl, sew = tab_idx[order], dst_local[order], ew[order]
    starts = {}
    pos = 0
    for c in range(NCORES):
        for w in range(NWIN):
            for h in range(2):
                n = int(counts[c, w, h])
                starts[(c, w, h)] = (pos, pos + n)
                pos += n

    schedule = []           # per group: (n_lo, n_hi, chunk_win list)
    total_chunks = 0
    for g in range(NGRP):
        wlo, whi = g * GRP, min((g + 1) * GRP, NWIN)
        chunk_win = []
        n_lo = n_hi = 0
        for w in range(wlo, whi):
            for _ in range(int(K[w, 0])):
                chunk_win.append(w - wlo)
                n_lo += 1
        for w in range(wlo, whi):
            for _ in range(int(K[w, 1])):
                chunk_win.append(w - wlo)
                n_hi += 1
        schedule.append((n_lo, n_hi, chunk_win))
        total_chunks += n_lo + n_hi

    n_lo_tot = sum(s[0] for s in schedule)
    n_hi_tot = sum(s[1] for s in schedule)

    per_core = []
    for c in range(NCORES):
        lo_idx = np.zeros(max(n_lo_tot, 1) * CHUNK, dtype=np.int16)
        hi_idx = np.zeros(max(n_hi_tot, 1) * CHUNK, dtype=np.int16)
        # compact selector encoding: per (slot, chunk) the dst column within
        # the window (or -1 for padding) and the edge weight.  The one-hot
        # selector tile is generated on-device as (iota == dcol) * ewc.
        dcol = np.full((CHUNK, total_chunks), -1.0, dtype=np.float32)
        ewc = np.zeros((CHUNK, total_chunks), dtype=np.float32)
        sel = (None if SEL_ONDEV else
               np.zeros((total_chunks, CHUNK, WIN), dtype=np.float32))
        ci = li = hi_i = 0
        for g in range(NGRP):
            wlo, whi = g * GRP, min((g + 1) * GRP, NWIN)
            for h in (0, 1):
                for w in range(wlo, whi):
                    a, b = starts[(c, w, h)]
                    es, ed, eww = ssrc[a:b], sdl[a:b], sew[a:b]
                    n = b - a
                    for k in range(int(K[w, h])):
                        s0, s1 = k * CHUNK, min((k + 1) * CHUNK, n)
                        cnt = max(0, s1 - s0)
                        if cnt > 0:
                            iv = es[s0:s1].astype(np.int16)
                            if h == 0:
                                lo_idx[li:li + cnt] = iv
                            else:
                                hi_idx[hi_i:hi_i + cnt] = iv
                            dcol[:cnt, ci] = ed[s0:s1] - w * WIN
                            ewc[:cnt, ci] = eww[s0:s1]
                            if sel is not None:
                                sel[ci, np.arange(cnt), ed[s0:s1] - w * WIN] = \
                                    eww[s0:s1]
                        if h == 0:
                            li += CHUNK
                        else:
                            hi_i += CHUNK
                        ci += 1

        def wrap(flat):
            ncols = len(flat) // 16
            out = np.empty((128, ncols), dtype=np.int16)
            v = flat.reshape(ncols, 16).T
            for g8 in range(8):
                out[g8 * 16:(g8 + 1) * 16] = v
            return out

        ent = dict(idx_lo=wrap(lo_idx), idx_hi=wrap(hi_idx))
        if SEL_ONDEV:
            ent.update(dcol=dcol, ewc=ewc)
        else:
            ent["sel"] = np.ascontiguousarray(
                sel.transpose(1, 0, 2).reshape(CHUNK, total_chunks * WIN)
            ).astype(MDT_NP)
        per_core.append(ent)
    return schedule, per_core


def make_inmaps(inp, per_core):
    """Build run_bass_kernel_spmd input maps from full inputs."""
    x = np.asarray(inp["x"], dtype=np.float32)
    wm, wg, gb = _pack_params(inp)
    in_maps = []
    for c in range(NCORES):
        pc = per_core[c]
        m = {
            "xT_in": np.ascontiguousarray(x[c * NPC:(c + 1) * NPC].T),
            "idx_lo": pc["idx_lo"], "idx_hi": pc["idx_hi"],
            "wm": wm, "wg": wg, "gb": gb,
        }
        if SEL_ONDEV:
            m["dcol"], m["ewc"] = pc["dcol"], pc["ewc"]
        else:
            m["sel"] = pc["sel"]
        in_maps.append(m)
    return in_maps


# --------------------------------------------------------------------------
# program builder
# --------------------------------------------------------------------------

def build_program(schedule, n_layers=NLAYERS, stage=99):
    total_chunks = sum(s[0] + s[1] for s in schedule)
    n_lo_tot = sum(s[0] for s in schedule)
    n_hi_tot = sum(s[1] for s in schedule)

    nc = bacc.Bacc("TRN2", target_bir_lowering=False, debug=False,
                   num_devices=NCORES, num_swdge_queues=2)

    xT_in = nc.dram_tensor("xT_in", [128, NPC], F32, kind="ExternalInput")
    idxlo_in = nc.dram_tensor("idx_lo", [128, max(n_lo_tot, 1) * 8], I16, kind="ExternalInput")
    idxhi_in = nc.dram_tensor("idx_hi", [128, max(n_hi_tot, 1) * 8], I16, kind="ExternalInput")
    if SEL_ONDEV:
        dcol_in = nc.dram_tensor("dcol", [128, total_chunks], F32, kind="ExternalInput")
        ewc_in = nc.dram_tensor("ewc", [128, total_chunks], F32, kind="ExternalInput")
    else:
        sel_in = nc.dram_tensor("sel", [128, total_chunks * WIN], MDT, kind="ExternalInput")
    wm_in = nc.dram_tensor("wm", [128, 4 * 128], F32, kind="ExternalInput")
    wg_in = nc.dram_tensor("wg", [128, 12 * 128], GDT, kind="ExternalInput")
    gb_in = nc.dram_tensor("gb", [128, 8], F32, kind="ExternalInput")
    outT = nc.dram_tensor("outT", [128, NPC], F32, kind="ExternalOutput")

    RA, RB = NCORES * HL, NCORES * (NPC - HL)
    m_own = nc.dram_tensor("m_own", [NPC, C], MDT)
    m_fullA = [nc.dram_tensor(f"m_fullA{i}", [RA, C], MDT, addr_space="Shared")
               for i in range(2)]
    m_fullB = [nc.dram_tensor(f"m_fullB{i}", [RB, C], MDT, addr_space="Shared")
               for i in range(2)]

    with tile.TileContext(nc) as tc:
        with (
            tc.tile_pool(name="res", bufs=1) as res,
            tc.tile_pool(name="gpool", bufs=4) as gpool,
            tc.tile_pool(name="spool", bufs=4) as spool,
            tc.tile_pool(name="aggp", bufs=2, space="PSUM") as aggp,
            tc.tile_pool(name="gatep", bufs=5, space="PSUM") as gatep,
            tc.tile_pool(name="mmp", bufs=1, space="PSUM") as mmp,
            tc.tile_pool(name="asb", bufs=2) as asb,
            tc.tile_pool(name="tsb", bufs=10) as tsb,
            tc.tile_pool(name="msb", bufs=4) as msb,
        ):
            # resident tiles
            xT = res.tile([128, NPC], F32)
            idxlo = res.tile([128, max(n_lo_tot, 1) * 8], I16)
            idxhi = res.tile([128, max(n_hi_tot, 1) * 8], I16)
            wm = res.tile([128, 4 * 128], F32)
            wg = res.tile([128, 12 * 128], GDT)
            gb = res.tile([128, 8], F32)
            nc.sync.dma_start(xT[:], xT_in[:])
            nc.sync.dma_start(idxlo[:], idxlo_in[:])
            nc.sync.dma_start(idxhi[:], idxhi_in[:])
            nc.sync.dma_start(wm[:], wm_in[:])
            nc.sync.dma_start(wg[:], wg_in[:])
            nc.sync.dma_start(gb[:], gb_in[:])
            if SEL_ONDEV:
                dcol = res.tile([128, total_chunks], F32)
                ewc = res.tile([128, total_chunks], F32)
                iota = res.tile([128, WIN], F32)
                nc.sync.dma_start(dcol[:], dcol_in[:])
                nc.sync.dma_start(ewc[:], ewc_in[:])
                nc.gpsimd.iota(iota[:], pattern=[[1, WIN]], base=0,
                               channel_multiplier=0,
                               allow_small_or_imprecise_dtypes=True)

            NKCH = (NPC + 127) // 128          # 49 node chunks for m-phase
            gidx = 0    # global gather counter: DMASW sem lane i pairs with
                        # queue i % 4, so queue must follow the same counter

            for L in range(n_layers):
                conv = 0 if L < 2 else 1
                wcol = (conv * 2 + (L % 2)) * 128
                sblk = conv * 6 * 128
                bcol = conv * 4
                relu = (L % 2 == 1) and (L < 17)
                mbufA = m_fullA[L % 2]
                mbufB = m_fullB[L % 2]
                KA = HL // 128              # m chunks feeding table A

                # ---- m-phase: m_own = x_own @ W; AG-A fires as soon as the
                # first HL rows are written so it overlaps the tail groups
                # of the previous layer. ----
                def m_chunk(k):
                    c0, c1 = k * 128, min((k + 1) * 128, NPC)
                    p = mmp.tile([128, 128], F32, tag="mm")
                    nc.tensor.matmul(p[:c1 - c0, :], xT[:, c0:c1],
                                     wm[:, wcol:wcol + 128],
                                     start=True, stop=True)
                    s = msb.tile([128, 128], MDT, tag="ms")
                    nc.scalar.copy(s[:c1 - c0, :], p[:c1 - c0, :])
                    nc.sync.dma_start(m_own[c0:c1, :], s[:c1 - c0, :])

                for k in range(KA):
                    m_chunk(k)
                if stage != 30:
                    nc.gpsimd.collective_compute(
                        "AllGather", mybir.AluOpType.bypass,
                        replica_groups=[list(range(NCORES))],
                        ins=[m_own[0:HL, :]], outs=[mbufA[:]],
                    )
                for k in range(KA, NKCH):
                    m_chunk(k)
                if stage != 30:
                    nc.gpsimd.collective_compute(
                        "AllGather", mybir.AluOpType.bypass,
                        replica_groups=[list(range(NCORES))],
                        ins=[m_own[HL:NPC, :]], outs=[mbufB[:]],
                    )
                m_lo = mbufA[:]
                m_hi = mbufB[:]
                if stage < 20:
                    continue

                # ---- message gather + selector matmuls + GRU, per group ----
                ci = 0      # global chunk cursor
                li = 0      # lo chunk cursor
                hi_i = 0    # hi chunk cursor
                for g in range(NGRP):
                    n_lo, n_hi, chunk_win = schedule[g]
                    g0 = g * GRP * WIN
                    gw = min(GRP * WIN, NPC - g0)
                    # One PSUM bank per half: start=True marks the whole 2KB
                    # bank pending-zero, so accumulation groups (windows)
                    # must be strictly sequential within a bank.
                    agg_lo = aggp.tile([128, 512], F32, tag="agg")
                    agg_hi = aggp.tile([128, 512], F32, tag="agg")
                    agg2 = [agg_lo, agg_hi]

                    # gather + matmul in sub-batches of <= SUB chunks
                    j = 0
                    while j < n_lo + n_hi:
                        if j < n_lo:
                            nch = min(SUB, n_lo - j)
                            h, idx_t, cur, table = 0, idxlo, li, m_lo
                            li += nch
                        else:
                            nch = min(SUB, n_lo + n_hi - j)
                            h, idx_t, cur, table = 1, idxhi, hi_i, m_hi
                            hi_i += nch
                        h0 = n_lo if h else 0               # half section start
                        h1 = n_lo + n_hi if h else n_lo     # half section end
                        gt = gpool.tile([128, SUB * 128], MDT, tag="g")
                        nc.gpsimd.dma_gather(
                            out_ap=gt[:, :nch * 128].rearrange(
                                "p (a b) -> p a b", b=128),
                            in_ap=table,
                            idxs_ap=idx_t[:, cur * 8:(cur + nch) * 8],
                            num_idxs=nch * 128, num_idxs_reg=nch * 128,
                            elem_size=C, single_packet=False,
                            queue_num=(li + hi_i) % 2,
                        )
                        if stage >= 21:
                            st = spool.tile([128, SUB * WIN], MDT, tag="s")
                            if SEL_MODE == 1:
                                # on-device selector: (iota == dcol) * ewc,
                                # alternating DVE / Pool to split the load
                                for q in range(nch):
                                    eng = nc.vector if q % 2 == 0 else nc.gpsimd
                                    cc = ci + j + q
                                    eng.tensor_scalar(
                                        st[:, q * WIN:(q + 1) * WIN], iota[:],
                                        dcol[:, cc:cc + 1], ewc[:, cc:cc + 1],
                                        mybir.AluOpType.is_equal,
                                        mybir.AluOpType.mult)
                            elif SEL_MODE == 2:
                                # batched on-device selector: broadcast APs,
                                # 2 elementwise ops per gather batch
                                c0 = ci + j
                                io3 = iota[:].rearrange(
                                    "p (o w) -> p o w", o=1
                                ).broadcast_to([128, nch, WIN])
                                dc3 = dcol[:, c0:c0 + nch].rearrange(
                                    "p (n o) -> p n o", o=1
                                ).broadcast_to([128, nch, WIN])
                                ew3 = ewc[:, c0:c0 + nch].rearrange(
                                    "p (n o) -> p n o", o=1
                                ).broadcast_to([128, nch, WIN])
                                st3 = st[:, :nch * WIN].rearrange(
                                    "p (n w) -> p n w", w=WIN)
                                eng = nc.vector if (li + hi_i) % 2 else nc.gpsimd
                                eng.tensor_tensor(
                                    st3, io3, dc3, mybir.AluOpType.is_equal)
                                eng.tensor_tensor(
                                    st3, st3, ew3, mybir.AluOpType.mult)
                            else:
                                nc.sync.dma_start(
                                    st[:, :nch * WIN],
                                    sel_in[:, (ci + j) * WIN:(ci + j + nch) * WIN])
                        for q in range(nch if stage >= 22 else 0):
                            wg_i = chunk_win[j + q]
                            first = (j + q == h0) or chunk_win[j + q - 1] != wg_i
                            last = (j + q == h1 - 1) or chunk_win[j + q + 1] != wg_i
                            nc.tensor.matmul(
                                agg2[h][:, wg_i * WIN:(wg_i + 1) * WIN],
                                gt[:, q * 128:(q + 1) * 128],
                                st[:, q * WIN:(q + 1) * WIN],
                                start=first, stop=last,
                            )
                        j += nch
                    ci += n_lo + n_hi

                    if stage < 23:
                        continue
                    # agg^T = lo + hi; a DVE op may read only ONE input from
                    # PSUM, so stage hi through the scalar engine first.  The
                    # sum is cast to MDT in the add so the gate matmuls run at
                    # 16-bit PE rate.
                    aggf = asb.tile([128, 512], F32, tag="aggf")
                    nc.scalar.copy(aggf[:, :gw], agg2[1][:, :gw])
                    aggs = asb.tile([128, 512], GDT, tag="aggs")
                    nc.vector.tensor_add(aggs[:, :gw], aggf[:, :gw],
                                         agg2[0][:, :gw])

                    if stage < 24:
                        continue
                    # ---- GRU for this 512-node group, feature-major ----
                    xg = xT[:, g0:g0 + gw]
                    if GATE_BF16:
                        xg16 = asb.tile([128, 512], GDT, tag="xg16")
                        nc.scalar.copy(xg16[:, :gw], xg)
                    else:
                        xg16 = None

                    def gate_mm(idx_ih, idx_hh, acc_two):
                        pt = gatep.tile([128, 512], F32, tag="gate")
                        nc.tensor.matmul(
                            pt[:, :gw],
                            wg[:, sblk + idx_ih * 128:sblk + (idx_ih + 1) * 128],
                            aggs[:, :gw], start=True, stop=not acc_two)
                        if acc_two:
                            nc.tensor.matmul(
                                pt[:, :gw],
                                wg[:, sblk + idx_hh * 128:sblk + (idx_hh + 1) * 128],
                                xg16[:, :gw] if GATE_BF16 else xg,
                                start=False, stop=True)
                        return pt

                    r_pre = gate_mm(0, 3, True)          # wihT_r, whhT_r
                    z_pre = gate_mm(1, 4, True)
                    i_n = gate_mm(2, None, False)        # wihT_n only
                    h_n = gatep.tile([128, 512], F32, tag="gate")
                    nc.tensor.matmul(h_n[:, :gw],
                                     wg[:, sblk + 5 * 128:sblk + 6 * 128],
                                     xg16[:, :gw] if GATE_BF16 else xg,
                                     start=True, stop=True)

                    if stage < 25:
                        continue
                    r = tsb.tile([128, 512], F32, tag="t")
                    nc.scalar.activation(r[:, :gw], r_pre[:, :gw],
                                         mybir.ActivationFunctionType.Sigmoid,
                                         bias=gb[:, bcol + 0:bcol + 1])
                    z = tsb.tile([128, 512], F32, tag="t")
                    nc.scalar.activation(z[:, :gw], z_pre[:, :gw],
                                         mybir.ActivationFunctionType.Sigmoid,
                                         bias=gb[:, bcol + 1:bcol + 1 + 1])
                    hnb = tsb.tile([128, 512], F32, tag="t")
                    nc.vector.tensor_scalar_add(hnb[:, :gw], h_n[:, :gw],
                                                gb[:, bcol + 3:bcol + 4])
                    rh = tsb.tile([128, 512], F32, tag="t")
                    nc.vector.tensor_mul(rh[:, :gw], r[:, :gw], hnb[:, :gw])
                    t1 = tsb.tile([128, 512], F32, tag="t")
                    nc.vector.tensor_add(t1[:, :gw], i_n[:, :gw], rh[:, :gw])
                    n_t = tsb.tile([128, 512], F32, tag="t")
                    nc.scalar.activation(n_t[:, :gw], t1[:, :gw],
                                         mybir.ActivationFunctionType.Tanh,
                                         bias=gb[:, bcol + 2:bcol + 3])
                    d = tsb.tile([128, 512], F32, tag="t")
                    nc.vector.tensor_sub(d[:, :gw], xg, n_t[:, :gw])
                    zd = tsb.tile([128, 512], F32, tag="t")
                    nc.vector.tensor_mul(zd[:, :gw], z[:, :gw], d[:, :gw])
                    nc.vector.tensor_add(xg, n_t[:, :gw], zd[:, :gw])
                    if relu:
                        nc.vector.tensor_scalar_max(xg, xg, 0.0)

            nc.sync.dma_start(outT[:], xT[:])

    nc.compile()
    return nc


# --------------------------------------------------------------------------
# entry point
# --------------------------------------------------------------------------

def _pack_params(inputs):
    wm = np.zeros((128, 4 * 128), dtype=np.float32)
    wg = np.zeros((128, 12 * 128), dtype=np.float32)
    gb = np.zeros((128, 8), dtype=np.float32)
    # wg is shipped at MDT precision (the gate matmuls run at 16-bit PE rate)
    for conv, tag in ((0, "1"), (1, "2")):
        w = np.asarray(inputs[f"w{tag}"], dtype=np.float32)
        wih = np.asarray(inputs[f"wih{tag}"], dtype=np.float32)
        whh = np.asarray(inputs[f"whh{tag}"], dtype=np.float32)
        bih = np.asarray(inputs[f"bih{tag}"], dtype=np.float32)
        bhh = np.asarray(inputs[f"bhh{tag}"], dtype=np.float32)
        for l in range(2):
            wm[:, (conv * 2 + l) * 128:(conv * 2 + l + 1) * 128] = w[l]
        for i, mat in enumerate((wih[0:128], wih[128:256], wih[256:384],
                                 whh[0:128], whh[128:256], whh[256:384])):
            wg[:, (conv * 6 + i) * 128:(conv * 6 + i + 1) * 128] = mat.T
        gb[:, conv * 4 + 0] = bih[0:128] + bhh[0:128]
        gb[:, conv * 4 + 1] = bih[128:256] + bhh[128:256]
        gb[:, conv * 4 + 2] = bih[256:384]
        gb[:, conv * 4 + 3] = bhh[256:384]
    return wm, wg.astype(GDT_NP), gb


_CACHE = {}


def kernel(**inputs):
    schedule, per_core = preprocess(inputs["edge_index"], inputs["edge_attr"])

    key = tuple((s[0], s[1]) for s in schedule)
    if key not in _CACHE:
        _CACHE[key] = build_program(schedule)
    nc = _CACHE[key]

    in_maps = make_inmaps(inputs, per_core)
    res = bass_utils.run_bass_kernel_spmd(nc, in_maps, list(range(NCORES)))
    out = np.concatenate(
        [res.results[c]["outT"].T for c in range(NCORES)], axis=0)
    return out.astype(np.float32)

